# revision 24
# baseline (speedup 1.0000x reference)
"""Trainium2 Bass kernel for CAConv2 (coordinate-attention + 3x3 conv block).

Shapes (hardcoded): x (8, 128, 128, 128) f32; data-parallel over batch,
one image per NeuronCore (8 cores).
"""

import numpy as np
import ml_dtypes

import concourse.bacc as bacc
import concourse.tile as tile
from concourse import mybir
from concourse.bass import ds
from concourse.bass_utils import run_bass_kernel_spmd
from concourse.tile_rust import add_dep_helper

BF16 = mybir.dt.bfloat16
F32 = mybir.dt.float32
C, H, W, MIP = 128, 128, 128, 8
WP = W + 4  # padded width: cols [2, 130) hold data, 0/1 and 130/131 are zero
HP = H + 2  # padded height: rows [1, 129) hold data
EPS = 1e-5
AF = mybir.ActivationFunctionType
ALU = mybir.AluOpType

_CACHE = {}


def build_nc():
    nc = bacc.Bacc()
    xp = nc.declare_dram_parameter("x", [C, H * W], BF16, isOutput=False)
    w1t = nc.declare_dram_parameter("w1t", [C, MIP], BF16, isOutput=False)
    wht = nc.declare_dram_parameter("wht", [MIP, C], BF16, isOutput=False)
    wwt = nc.declare_dram_parameter("wwt", [MIP, C], BF16, isOutput=False)
    # wct[i, k, o] = wc[o, i, k//3, k%3]
    wct = nc.declare_dram_parameter("wct", [C, 9 * C], BF16, isOutput=False)
    # p8 cols: 0: s1/6, 1: t1f/6, 2: s1, 3: t1f+3   (t1f = s1*b1 + be1 - m1*s1)
    p8 = nc.declare_dram_parameter("p8", [MIP, 4], F32, isOutput=False)
    # p128 cols: 0: bh, 1: bw, 2: s2, 3: b2 (= bc*s2 + be2 - m2*s2)
    p128 = nc.declare_dram_parameter("p128", [C, 4], F32, isOutput=False)
    outp = nc.declare_dram_parameter("out", [C, H, W], F32, isOutput=True)

    c1, c2, c3 = 7.0 / 128, 3.0 / 128, 1.0 / 128

    with tile.TileContext(nc) as tc:
        with (
            tc.tile_pool(name="sing", bufs=1) as sing,
            tc.tile_pool(name="pp", bufs=2) as pp,
            tc.tile_pool(name="small", bufs=1) as small,
        ):
            xs = sing.tile([C, H * W], BF16)
            ug = sing.tile([C, HP, WP], BF16)
            s32 = sing.tile([C, H, 4], F32)

            # weights/params ride the sync ring
            w1t_sb = sing.tile([C, MIP], BF16)
            nc.sync.dma_start(out=w1t_sb, in_=w1t[:, :])
            # x chunks all on the gpsimd SWDGE ring: descriptors drain in
            # issue order -> staggered completion at full BW. gpsimd does
            # NOTHING else during the input window (tensor work on the Q7
            # cores starves descriptor generation).
            XCH = [(0, 8), (8, 24), (32, 32), (64, 16), (80, 16), (96, 16),
                   (112, 8), (120, 8)]
            for r0, nr in XCH:
                nc.gpsimd.dma_start(
                    out=xs[:, ds(r0 * W, nr * W)],
                    in_=xp[:, ds(r0 * W, nr * W)],
                )
            wht_sb = sing.tile([MIP, C], BF16)
            nc.sync.dma_start(out=wht_sb, in_=wht[:, :])
            wwt_sb = sing.tile([MIP, C], BF16)
            nc.sync.dma_start(out=wwt_sb, in_=wwt[:, :])
            p8_sb = sing.tile([MIP, 4], F32)
            nc.sync.dma_start(out=p8_sb, in_=p8[:, :])
            p128_sb = sing.tile([C, 4], F32)
            nc.sync.dma_start(out=p128_sb, in_=p128[:, :])
            wct_sb = sing.tile([C, 9, C], BF16)
            nc.sync.dma_start(out=wct_sb, in_=wct.rearrange("i (k o) -> i k o", k=9))

            # conv padding border of ug (DVE is idle this early)
            nc.vector.memset(ug[:, 0, :], 0.0)
            nc.vector.memset(ug[:, HP - 1, :], 0.0)
            nc.vector.memset(ug[:, 1 : HP - 1, 0:2], 0.0)
            nc.vector.memset(ug[:, 1 : HP - 1, WP - 2 : WP], 0.0)

            # preload ACT function tables off the critical path
            dummy = small.tile([C, 2], F32)
            nc.vector.memset(dummy, 0.0)
            dump = small.tile([C, 2], F32)
            for fn in (AF.Silu, AF.Sigmoid):
                nc.scalar.activation(dump, dummy, fn, bias=0.0, scale=1.0)

            with tc.tile_pool(name="psA", bufs=1, space="PSUM") as psA:
                # x_w row ranges: [0,32) w=c1, [32,64) w=c2, [64,96) and
                # [96,128) w=c3; two-row matmuls accumulate onto (8, 2, W)
                psxw = [
                    psA.tile([MIP, 2, W], F32, name=f"xw{r}", tag=f"xw{r}")
                    for r in range(4)
                ]
                ps_yh = psA.tile([MIP, H], F32, tag="yh")
                ps_ah = psA.tile([C, H], F32, tag="ah")
                ah_sb = small.tile([C, H], BF16)
                xq = small.tile([MIP, W], F32)  # x_w combine accumulator

                def emit_tree(r0, nr):
                    # 32-col segment sums for rows [r0, r0+nr)
                    eng = nc.vector
                    xc = xs[:, ds(r0 * W, nr * W)].rearrange(
                        "p (y q s) -> p y q s", q=4, s=32
                    )
                    t1 = pp.tile([C, 16, 4, 16], BF16, tag="t1")
                    eng.tensor_add(
                        t1[:, :nr], xc[:, :, :, 0:16], xc[:, :, :, 16:32]
                    )
                    t2 = pp.tile([C, 16, 4, 8], BF16, tag="t2")
                    eng.tensor_add(t2[:, :nr], t1[:, :nr, :, 0:8], t1[:, :nr, :, 8:16])
                    t3 = pp.tile([C, 16, 4, 4], BF16, tag="t3")
                    eng.tensor_add(t3[:, :nr], t2[:, :nr, :, 0:4], t2[:, :nr, :, 4:8])
                    t4 = pp.tile([C, 16, 4, 2], BF16, tag="t4")
                    eng.tensor_add(t4[:, :nr], t3[:, :nr, :, 0:2], t3[:, :nr, :, 2:4])
                    sl = s32[:, ds(r0, nr), :]
                    eng.tensor_add(sl, t4[:, :nr, :, 0], t4[:, :nr, :, 1])

                def bn_hswish(src, dst, n):
                    # dst = h_swish(s1*src + t1f) for an (MIP, n) slice
                    z6 = pp.tile([MIP, n], F32, tag="bn_z6")
                    nc.vector.tensor_scalar(
                        out=z6, in0=src, scalar1=p8_sb[:, 0:1],
                        scalar2=p8_sb[:, 1:2], op0=ALU.mult, op1=ALU.add,
                    )
                    r = pp.tile([MIP, n], F32, tag="bn_r")
                    nc.vector.tensor_scalar(
                        out=r, in0=z6, scalar1=6.0, scalar2=3.0,
                        op0=ALU.mult, op1=ALU.add,
                    )
                    rc = pp.tile([MIP, n], F32, tag="bn_rc")
                    nc.vector.tensor_scalar(
                        out=rc, in0=r, scalar1=0.0, scalar2=6.0,
                        op0=ALU.max, op1=ALU.min,
                    )
                    nc.vector.tensor_mul(dst, z6, rc)

                def xh_block(rlo, rhi):
                    # combine s32 rows [rlo, rhi) -> pooled -> yh -> bn -> ah
                    n = rhi - rlo
                    slh = s32[:, ds(rlo, n), :]
                    tmpA = pp.tile([C, n], F32, tag="tmpA")
                    nc.vector.tensor_add(tmpA, slh[:, :, 2], slh[:, :, 3])
                    m0 = pp.tile([C, n], F32, tag="m0")
                    nc.vector.tensor_scalar_mul(m0, slh[:, :, 0], c1)
                    m1 = pp.tile([C, n], F32, tag="m1")
                    nc.vector.scalar_tensor_tensor(
                        out=m1, in0=slh[:, :, 1], scalar=c2, in1=m0,
                        op0=ALU.mult, op1=ALU.add,
                    )
                    xhp = pp.tile([C, n], BF16, tag="xhp")
                    nc.vector.scalar_tensor_tensor(
                        out=xhp, in0=tmpA, scalar=c3, in1=m1,
                        op0=ALU.mult, op1=ALU.add,
                    )
                    nc.tensor.matmul(
                        ps_yh[:, ds(rlo, n)], w1t_sb, xhp, start=True, stop=True
                    )
                    xh_sh = pp.tile([MIP, n], BF16, tag="xh_sh")
                    bn_hswish(ps_yh[:, ds(rlo, n)], xh_sh, n)
                    nc.tensor.matmul(
                        ps_ah[:, ds(rlo, n)], wht_sb, xh_sh, start=True, stop=True
                    )
                    nc.scalar.activation(
                        ah_sb[:, ds(rlo, n)], ps_ah[:, ds(rlo, n)],
                        AF.Sigmoid, bias=p128_sb[:, 0:1], scale=1.0,
                    )

                def xw_precombine(r, first):
                    # accumulate range r (weight cf) into xq: 2 STT ops
                    cf = (c1, c2, c3, c3)[r]
                    if first:
                        nc.vector.tensor_scalar_mul(xq, psxw[r][:, 0, :], cf)
                    else:
                        nc.vector.scalar_tensor_tensor(
                            out=xq, in0=psxw[r][:, 0, :], scalar=cf, in1=xq,
                            op0=ALU.mult, op1=ALU.add,
                        )
                    nc.vector.scalar_tensor_tensor(
                        out=xq, in0=psxw[r][:, 1, :], scalar=cf, in1=xq,
                        op0=ALU.mult, op1=ALU.add,
                    )

                def gate_rows(rlo, rhi):
                    # ug rows = x * a_h[c,y] * a_w[c,x], fused per row
                    for y in range(rlo, rhi):
                        nc.vector.scalar_tensor_tensor(
                            out=ug[:, 1 + y, 2 : 2 + W],
                            in0=xs[:, ds(y * W, W)],
                            scalar=ah_sb[:, y : y + 1],
                            in1=aw_sb,
                            op0=ALU.mult,
                            op1=ALU.mult,
                        )

                # ---- chunk-chasing: row matmuls + trees ----
                tree_done = 0
                for r0, nr in XCH:
                    for b in range(0, nr, 2):
                        row = r0 + b
                        r = row // 32
                        nc.tensor.matmul(
                            psxw[r],
                            w1t_sb,
                            xs[:, ds(row * W, 2 * W)],
                            start=(row % 32 == 0),
                            stop=(row % 32 == 30),
                        )
                    while tree_done < r0 + nr:
                        n = min(16, r0 + nr - tree_done)
                        emit_tree(tree_done, n)
                        tree_done += n
                    if r0 + nr == 32:
                        xw_precombine(0, True)
                    elif r0 + nr == 64:
                        xw_precombine(1, False)
                        xh_block(0, 64)  # block A
                    elif r0 + nr == 96:
                        xw_precombine(2, False)

                # ---- a_w: the only chain on the critical path ----
                xw_precombine(3, False)
                xw_s = small.tile([MIP, W], BF16)
                bn_hswish(xq, xw_s, W)
                ps_aw = psA.tile([C, W], F32, tag="aw")
                nc.tensor.matmul(ps_aw, wwt_sb, xw_s, start=True, stop=True)
                aw_sb = small.tile([C, W], BF16)
                nc.scalar.activation(
                    aw_sb, ps_aw, AF.Sigmoid, bias=p128_sb[:, 1:2], scale=1.0
                )

                # finish the x_h path for the remaining rows, then gate
                xh_block(64, 128)
                gate_rows(0, 128)

            # ---- 3x3 conv + BN2 + SiLU ----
            with (
                tc.tile_pool(name="psB", bufs=3, space="PSUM") as psB,
                tc.tile_pool(name="obp", bufs=3) as obp,
            ):
                for rb in range(H // 4):
                    pso = psB.tile([C, 4, W], F32, tag="pso")
                    for k in range(9):
                        dy, dx = k // 3, k % 3
                        nc.tensor.matmul(
                            pso,
                            wct_sb[:, k, :],
                            ug[:, 4 * rb + dy : 4 * rb + dy + 4, 1 + dx : 1 + dx + W],
                            start=(k == 0),
                            stop=(k == 8),
                        )
                    ob = obp.tile([C, 4, W], F32, tag="ob")
                    nc.scalar.activation(
                        ob, pso, AF.Silu, bias=p128_sb[:, 3:4], scale=p128_sb[:, 2:3]
                    )
                    nc.sync.dma_start(out=outp[:, 4 * rb : 4 * rb + 4, :], in_=ob)

    nc.compile()
    return nc


def prep_inputs(x, w1, b1, g1, be1, m1, v1, wh, bh, ww, bw, wc, bc, g2, be2, m2, v2):
    """Host-side prep: per-core input maps (weights replicated)."""
    bf = ml_dtypes.bfloat16
    N = x.shape[0]
    s1 = (g1 / np.sqrt(v1 + EPS)).astype(np.float64)
    t1f = s1 * b1 + be1 - m1 * s1
    p8 = np.stack([s1 / 6.0, t1f / 6.0, s1, t1f + 3.0], axis=1).astype(np.float32)
    s2 = (g2 / np.sqrt(v2 + EPS)).astype(np.float64)
    b2 = bc * s2 + be2 - m2 * s2
    p128 = np.stack([bh, bw, s2, b2], axis=1).astype(np.float32)
    shared = {
        "w1t": np.ascontiguousarray(w1.T).astype(bf),            # (C, MIP)
        "wht": np.ascontiguousarray(wh.T).astype(bf),            # (MIP, C)
        "wwt": np.ascontiguousarray(ww.T).astype(bf),            # (MIP, C)
        "wct": np.ascontiguousarray(
            np.transpose(wc, (1, 2, 3, 0)).reshape(C, 9 * C)
        ).astype(bf),                                            # [i, (ky kx), o]
        "p8": p8,
        "p128": p128,
    }
    in_maps = []
    for n in range(N):
        m = dict(shared)
        m["x"] = np.ascontiguousarray(x[n].reshape(C, H * W)).astype(bf)
        in_maps.append(m)
    return in_maps


def run(inputs, trace=False):
    if "nc" not in _CACHE:
        _CACHE["nc"] = build_nc()
    nc = _CACHE["nc"]
    in_maps = prep_inputs(**inputs)
    res = run_bass_kernel_spmd(nc, in_maps, core_ids=list(range(8)), trace=trace)
    out = np.stack([np.asarray(res.results[i]["out"]) for i in range(8)], axis=0)
    return out.astype(np.float32), res


def kernel(**inputs) -> np.ndarray:
    out, _ = run(inputs, trace=False)
    return out
